# revision 1
# baseline (speedup 1.0000x reference)
"""AFNO2D (channel-first) Trainium2 kernel.

out = x + irfft2( softshrink(mlp2(leaky(mlp1(rfft2(x))))) * rfft2(x) )
with block-diagonal complex MLPs over 8 channel blocks of 96.

Sharding: block-parallel — core k owns spectral block k (96 channels), fully
independent, zero collectives. All DFTs are dense bf16 matmuls on the
TensorEngine with ortho scaling folded into host-precomputed twiddle
matrices. Biases are folded into augmented stationaries via an ones-row.
Residual add in f32 on device.

PSUM rule: matmul start=True clears has_written for the WHOLE bank, so each
PSUM tile gets exactly one start=True (its first matmul); all later matmuls
use start=False (fresh ranges overwrite, accumulation ranges add).

Hardcoded shapes: x [4,768,128,128] f32, w1/w2 [2,8,96,96], b1/b2 [2,8,96].
"""

import os
import numpy as np
import ml_dtypes

B, C, H, W = 4, 768, 128, 128
NBLK, BS = 8, 96          # spectral blocks, channels per core
WF = 65                   # rfft size along W
LAM = 0.01                # softshrink threshold
NS = 0.1                  # leaky relu negative slope

BF16 = ml_dtypes.bfloat16

LAST_RESULT = {}          # diagnostics (exec_time_ns) for the test harness


def _twiddles():
    n = 128
    wv = np.arange(n)[:, None].astype(np.float64)
    jv = np.arange(n)[None, :].astype(np.float64)
    ang = 2.0 * np.pi * wv * jv / n  # [128,128]

    # S1 moving operand [w, 130]: cols 0..64 cos/n ; cols 65..129 -sin/n
    # (imag cols 65 and 129 i.e. wf=0,64 are exactly zero)
    fw = np.zeros((n, 130), np.float64)
    fw[:, :WF] = np.cos(ang[:, :WF]) / n
    fw[:, WF + 1:WF + 64] = -np.sin(ang[:, 1:64]) / n

    s2c = np.cos(ang)       # [h, hf] symmetric
    s2s = np.sin(ang)
    s2sn = -s2s

    # final irfft_W moving operands
    alpha = np.full((WF, 1), 2.0)
    alpha[0, 0] = 1.0
    alpha[64, 0] = 1.0
    fic = alpha * np.cos(2.0 * np.pi * np.arange(WF)[:, None] * np.arange(n)[None, :] / n) / n
    fis = -2.0 * np.sin(2.0 * np.pi * np.arange(1, 64)[:, None] * np.arange(n)[None, :] / n) / n

    ident = np.eye(n)
    return (fw.astype(BF16), s2c.astype(BF16), s2s.astype(BF16),
            s2sn.astype(BF16), fic.astype(BF16), fis.astype(BF16),
            ident.astype(BF16))


def _build():
    import concourse.mybir as mybir
    import concourse.tile as tile
    from concourse import bacc

    dt = mybir.dt
    AF = mybir.ActivationFunctionType
    ALU = mybir.AluOpType

    nc = bacc.Bacc("TRN2", target_bir_lowering=False, debug=False)

    xt = nc.declare_dram_parameter("xt", [B, W, BS, H], dt.bfloat16, isOutput=False)
    xres = nc.declare_dram_parameter("xres", [B, BS, H, W], dt.float32, isOutput=False)
    out = nc.declare_dram_parameter("out", [B, BS, H, W], dt.float32, isOutput=True)

    fw_d = nc.declare_dram_parameter("fw", [128, 130], dt.bfloat16, isOutput=False)
    s2c_d = nc.declare_dram_parameter("s2c", [128, 128], dt.bfloat16, isOutput=False)
    s2s_d = nc.declare_dram_parameter("s2s", [128, 128], dt.bfloat16, isOutput=False)
    s2sn_d = nc.declare_dram_parameter("s2sn", [128, 128], dt.bfloat16, isOutput=False)
    fic_d = nc.declare_dram_parameter("fic", [WF, 128], dt.bfloat16, isOutput=False)
    fis_d = nc.declare_dram_parameter("fis", [63, 128], dt.bfloat16, isOutput=False)
    id_d = nc.declare_dram_parameter("ident", [128, 128], dt.bfloat16, isOutput=False)

    # augmented stationaries: w1ra=[w1r;b1r] etc (ones-row bias fold)
    wnames = ("w1ra", "w1ia", "w1r", "w1in", "w2ra", "w2ia", "w2r", "w2in")
    wshapes = {"w1ra": BS + 1, "w1ia": BS + 1, "w2ra": BS + 1, "w2ia": BS + 1,
               "w1r": BS, "w1in": BS, "w2r": BS, "w2in": BS}
    wds = {nm: nc.declare_dram_parameter(nm, [wshapes[nm], 128], dt.bfloat16,
                                         isOutput=False) for nm in wnames}

    dbg = bool(int(os.environ.get("AFNO_DEBUG", "0")))
    if dbg:
        d_u1 = nc.declare_dram_parameter("d_u1", [128, BS, 130], dt.bfloat16, isOutput=True)
        d_ub = nc.declare_dram_parameter("d_ub", [128, BS, 130], dt.bfloat16, isOutput=True)
        d_utr = nc.declare_dram_parameter("d_utr", [BS, 2 * WF * 128], dt.bfloat16, isOutput=True)
        d_yt = nc.declare_dram_parameter("d_yt", [128, BS, 130], dt.bfloat16, isOutput=True)

    with tile.TileContext(nc) as tc:
        with (
            tc.tile_pool(name="consts", bufs=1) as consts,
            tc.tile_pool(name="xts", bufs=3) as xts_p,
            tc.tile_pool(name="u1", bufs=1) as u1_p,
            tc.tile_pool(name="big", bufs=2) as big_p,
            tc.tile_pool(name="utr", bufs=1) as utr_p,
            tc.tile_pool(name="chunk", bufs=4) as chunk_p,
            tc.tile_pool(name="gtmp", bufs=4) as gtmp_p,
            tc.tile_pool(name="vb", bufs=4) as vb_p,
            tc.tile_pool(name="vtb", bufs=3) as vtb_p,
            tc.tile_pool(name="iot", bufs=3) as iot_p,
            tc.tile_pool(name="pS", bufs=3, space="PSUM") as pS,
            tc.tile_pool(name="pT", bufs=2, space="PSUM") as pT,
            tc.tile_pool(name="pM1", bufs=2, space="PSUM") as pM1,
            tc.tile_pool(name="pM2", bufs=1, space="PSUM") as pM2,
        ):
            # ---------------- constants ----------------
            def cload(dparam, shape, dtype, tag):
                t = consts.tile(shape, dtype, tag=tag, name=tag)
                nc.sync.dma_start(out=t[:], in_=dparam[:, :])
                return t

            fw = cload(fw_d, [128, 130], dt.bfloat16, "fw")
            s2c = cload(s2c_d, [128, 128], dt.bfloat16, "s2c")
            s2s = cload(s2s_d, [128, 128], dt.bfloat16, "s2s")
            s2sn = cload(s2sn_d, [128, 128], dt.bfloat16, "s2sn")
            fic = cload(fic_d, [WF, 128], dt.bfloat16, "fic")
            fis = cload(fis_d, [63, 128], dt.bfloat16, "fis")
            ident = cload(id_d, [128, 128], dt.bfloat16, "ident")
            wt = {nm: cload(wds[nm], [wshapes[nm], 128], dt.bfloat16, nm)
                  for nm in wnames}

            for b in range(B):
                # ---------- load x transposed (two halves) ----------
                xh = []
                for hh in range(2):
                    t = xts_p.tile([128, 48, 128], dt.bfloat16, tag="xts", name="xts")
                    nc.sync.dma_start(out=t[:], in_=xt[b, :, hh * 48:(hh + 1) * 48, :])
                    xh.append(t)

                # ---------- S1: rfft along W (data-stationary) ----------
                u1 = u1_p.tile([128, BS, 130], dt.bfloat16, tag="u1")
                for g in range(BS // 3):
                    ps = pS.tile([128, 3, 130], dt.float32, tag="pS", name="ps1")
                    for k in range(3):
                        c = 3 * g + k
                        lhs = xh[c // 48][:, c % 48, :]
                        nc.tensor.matmul(ps[:, k, :], lhs, fw,
                                         start=(k == 0), stop=(k == 2),
                                         skip_group_check=True)
                    nc.any.tensor_copy(u1[:, 3 * g:3 * g + 3, :], ps[:, :, :])
                if dbg and b == 0:
                    nc.sync.dma_start(out=d_u1[:, :, :], in_=u1[:, :, :])
                # ---------- S2: DFT along H, data-stationary per frequency ----
                # lhsT = u1[:, :, j]  ([h=128, c=96] slab, strided), moving =
                # cos/sin matrices. Output lands directly as [c, hf] slabs in
                # utr (mix-ready layout) — no PE transposes, no preT.
                utr = utr_p.tile([BS + 1, 2 * WF * 128], dt.bfloat16, tag="utr")
                nc.gpsimd.memset(utr[BS:BS + 1, :], 1.0)   # ones-row (bias fold)
                RB = WF * 128   # imag block offset in utr
                import concourse.bass as _bassS
                _ubase = utr[0:BS, :]
                for j0 in range(0, WF, 2):
                    jj = [j for j in (j0, j0 + 1) if j < WF]
                    nj = len(jj)
                    ps = pS.tile([BS, 2, nj, 128], dt.float32, tag="pS",
                                 name="psw")
                    for q, j in enumerate(jj):
                        lr = u1[:, :, j]
                        li = u1[:, :, 65 + j]
                        nc.tensor.matmul(ps[:, 0, q, :], lr, s2c,
                                         start=(q == 0), stop=False,
                                         skip_group_check=True)
                        nc.tensor.matmul(ps[:, 1, q, :], lr, s2sn,
                                         start=False, stop=False,
                                         skip_group_check=True)
                        nc.tensor.matmul(ps[:, 0, q, :], li, s2s,
                                         start=False, stop=False,
                                         skip_group_check=True)
                        nc.tensor.matmul(ps[:, 1, q, :], li, s2c,
                                         start=False, stop=(q == nj - 1),
                                         skip_group_check=True)
                    dstS = _bassS.AP(
                        tensor=_ubase.tensor, offset=_ubase.offset + j0 * 128,
                        ap=[_ubase.ap[0], [RB, 2], [128, nj], [1, 128]])
                    nc.any.tensor_copy(dstS, ps[:, :, :, :])
                if dbg and b == 0:
                    nc.sync.dma_start(out=d_utr[:, :], in_=utr[0:BS, :])

                # ---------- mix1 -> leaky -> mix2 -> shrink -> gate -> backT --
                # 2-stage software pipeline: stage A (mix1+Prelu) of chunk i+1
                # is emitted before stage B (mix2..backT) of chunk i so the PE
                # stream never waits on ACT/DVE results of the current chunk.
                yt = big_p.tile([128, BS, 130], dt.bfloat16, tag="big", name="yt")
                nch = (WF * 128) // 256
                chunks = [(ci * 256, 256) for ci in range(nch)] + \
                         [(nch * 256, WF * 128 - nch * 256)]

                import concourse.bass as _bass

                def mix_stageA(off, sz):
                    ura = utr[0:BS + 1, off:off + sz]          # [97, sz] w/ ones
                    ui = utr[0:BS, WF * 128 + off:WF * 128 + off + sz]
                    p1 = pM1.tile([128, 512], dt.float32, tag="pM1", name="p1")
                    nc.tensor.matmul(p1[:, 0:sz], wt["w1ra"], ura,
                                     start=True, stop=False, skip_group_check=True)
                    nc.tensor.matmul(p1[:, 256:256 + sz], wt["w1ia"], ura,
                                     start=False, stop=False, skip_group_check=True)
                    nc.tensor.matmul(p1[:, 0:sz], wt["w1in"], ui,
                                     start=False, stop=False, skip_group_check=True)
                    nc.tensor.matmul(p1[:, 256:256 + sz], wt["w1r"], ui,
                                     start=False, stop=True, skip_group_check=True)
                    o1 = chunk_p.tile([BS + 1, 512], dt.bfloat16, tag="o1",
                                      name="o1")
                    nc.gpsimd.memset(o1[BS:BS + 1, :], 1.0)
                    nc.scalar.activation(o1[0:BS, :], p1[0:BS, :], AF.Prelu,
                                         bias=0.0, scale=1.0, alpha=NS)
                    return (off, sz, o1)

                def mix_stageB(st):
                    off, sz, o1 = st
                    ur = utr[0:BS, off:off + sz]
                    ui = utr[0:BS, WF * 128 + off:WF * 128 + off + sz]
                    p2 = pM2.tile([128, 512], dt.float32, tag="pM2", name="p2")
                    nc.tensor.matmul(p2[:, 0:sz], wt["w2ra"], o1[0:BS + 1, 0:sz],
                                     start=True, stop=False, skip_group_check=True)
                    nc.tensor.matmul(p2[:, 256:256 + sz], wt["w2ia"],
                                     o1[0:BS + 1, 0:sz],
                                     start=False, stop=False, skip_group_check=True)
                    nc.tensor.matmul(p2[:, 0:sz], wt["w2in"], o1[0:BS, 256:256 + sz],
                                     start=False, stop=False, skip_group_check=True)
                    nc.tensor.matmul(p2[:, 256:256 + sz], wt["w2r"],
                                     o1[0:BS, 256:256 + sz],
                                     start=False, stop=True, skip_group_check=True)
                    # softshrink straight from PSUM: s = p2 - clamp(p2)
                    cl = chunk_p.tile([BS, 512], dt.bfloat16, tag="cl", name="cl")
                    nc.vector.tensor_scalar(cl[:, :], p2[0:BS, :], -LAM, LAM,
                                            ALU.max, ALU.min)
                    sh = chunk_p.tile([BS, 512], dt.bfloat16, tag="sh", name="sh")
                    nc.vector.tensor_sub(sh[:, :], p2[0:BS, :], cl[:, :])
                    # gate: y = s * U (complex elementwise)
                    _ubase = utr[0:BS, :]
                    u_pair = _bass.AP(
                        tensor=_ubase.tensor, offset=_ubase.offset + off,
                        ap=[_ubase.ap[0], [WF * 128, 2], [1, sz]])
                    _sbase = sh[:, :]
                    sr_rep = _bass.AP(
                        tensor=_sbase.tensor, offset=_sbase.offset,
                        ap=[_sbase.ap[0], [0, 2], [1, sz]])
                    si_rep = _bass.AP(
                        tensor=_sbase.tensor, offset=_sbase.offset + 256,
                        ap=[_sbase.ap[0], [0, 2], [1, sz]])
                    ta = gtmp_p.tile([BS, 2, 256], dt.bfloat16, tag="ta", name="ta")
                    tb = gtmp_p.tile([BS, 2, 256], dt.bfloat16, tag="tb", name="tb")
                    yg = gtmp_p.tile([BS, 512], dt.bfloat16, tag="yg", name="yg")
                    nc.vector.tensor_mul(ta[:, :, 0:sz], sr_rep, u_pair)
                    nc.gpsimd.tensor_mul(tb[:, :, 0:sz], si_rep, u_pair)
                    nc.vector.tensor_sub(yg[:, 0:sz], ta[:, 0, 0:sz], tb[:, 1, 0:sz])
                    nc.gpsimd.tensor_add(yg[:, 256:256 + sz], ta[:, 1, 0:sz],
                                         tb[:, 0, 0:sz])
                    # backT into yt [hf, (c, 130)]
                    nsl = sz // 128
                    ps = pT.tile([128, 512], dt.bfloat16, tag="pT", name="psb")
                    for sl in range(nsl):
                        nc.tensor.transpose(ps[:, sl * 96:(sl + 1) * 96],
                                            yg[:, sl * 128:(sl + 1) * 128],
                                            ident[0:BS, 0:BS])
                        nc.tensor.transpose(ps[:, (nsl + sl) * 96:(nsl + sl + 1) * 96],
                                            yg[:, 256 + sl * 128:256 + (sl + 1) * 128],
                                            ident[0:BS, 0:BS])
                    j0 = off // 128
                    _ybase = yt[:, :, :]
                    dst = _bass.AP(
                        tensor=_ybase.tensor, offset=_ybase.offset + j0,
                        ap=[_ybase.ap[0], [65, 2], [1, nsl], [130, BS]])
                    nc.any.tensor_copy(
                        dst, ps[:, 0:2 * nsl * 96].rearrange(
                            "p (h j c) -> p h j c", h=2, j=nsl))

                prev = None
                for (off, sz) in chunks:
                    cur = mix_stageA(off, sz)
                    if prev is not None:
                        mix_stageB(prev)
                    prev = cur
                mix_stageB(prev)
                if dbg and b == 0:
                    nc.sync.dma_start(out=d_yt[:, :, :], in_=yt[:, :, :])

                # ---------- iDFT along H + final irfft_W + residual ----------
                # 2-stage pipeline over groups of 2 psum tiles (6 channels)
                def idft_stageA(tl):
                    pss = [pS.tile([128, 390], dt.float32, tag="pS", name="psv")
                           for _ in tl]
                    for ps, t in zip(pss, tl):     # cos.Yr -> Vr
                        nc.tensor.matmul(ps[:, 0:195], s2c,
                                         yt[:, 3 * t:3 * t + 3, 0:65],
                                         start=True, stop=False,
                                         skip_group_check=True)
                    for ps, t in zip(pss, tl):     # -sin.Yi -> Vr (acc)
                        nc.tensor.matmul(ps[:, 0:195], s2sn,
                                         yt[:, 3 * t:3 * t + 3, 65:130],
                                         start=False, stop=False,
                                         skip_group_check=True)
                    for ps, t in zip(pss, tl):     # sin.Yr -> Vi (full 65)
                        nc.tensor.matmul(ps[:, 195:390], s2s,
                                         yt[:, 3 * t:3 * t + 3, 0:65],
                                         start=False, stop=False,
                                         skip_group_check=True)
                    for ps, t in zip(pss, tl):     # cos.Yi -> Vi (acc)
                        nc.tensor.matmul(ps[:, 195:390], s2c,
                                         yt[:, 3 * t:3 * t + 3, 65:130],
                                         start=False, stop=True,
                                         skip_group_check=True)
                    items = []
                    for ps, t in zip(pss, tl):
                        vb = vb_p.tile([128, 390], dt.bfloat16, tag="vb", name="vb")
                        nc.any.tensor_copy(vb[:, :], ps[:, :])
                        items.append((vb, t))
                    return items

                def idft_stageB(items):
                    for vb, t in items:
                        psr = pT.tile([128, 512], dt.bfloat16, tag="pT", name="psr")
                        for k in range(3):
                            nc.tensor.transpose(psr[0:WF, k * 128:(k + 1) * 128],
                                                vb[:, 65 * k:65 * k + 65], ident)
                        psi = pT.tile([128, 512], dt.bfloat16, tag="pT", name="psi")
                        for k in range(3):
                            nc.tensor.transpose(psi[0:63, k * 128:(k + 1) * 128],
                                                vb[:, 195 + 65 * k + 1:195 + 65 * k + 64],
                                                ident)
                        vtr = vtb_p.tile([WF, 384], dt.bfloat16, tag="vtr", name="vtr")
                        nc.any.tensor_copy(vtr[:, :], psr[0:WF, 0:384])
                        vti = vtb_p.tile([63, 384], dt.bfloat16, tag="vti", name="vti")
                        nc.any.tensor_copy(vti[:, :], psi[0:63, 0:384])
                        po = pT.tile([128, 384], dt.float32, tag="pT", name="po")
                        for k in range(3):
                            nc.tensor.matmul(po[:, k * 128:(k + 1) * 128],
                                             vtr[:, k * 128:(k + 1) * 128], fic,
                                             start=(k == 0), stop=False,
                                             skip_group_check=True)
                            nc.tensor.matmul(po[:, k * 128:(k + 1) * 128],
                                             vti[:, k * 128:(k + 1) * 128], fis,
                                             start=False, stop=(k == 2),
                                             skip_group_check=True)
                        c0 = 3 * t
                        xr = iot_p.tile([128, 3, 128], dt.float32, tag="xr",
                                        name="xr")
                        nc.sync.dma_start(
                            out=xr[:, :, :],
                            in_=xres[b, c0:c0 + 3, :, :].rearrange(
                                "c h w -> h c w"))
                        ot = iot_p.tile([128, 3, 128], dt.float32, tag="ot",
                                        name="ot")
                        nc.vector.tensor_add(
                            ot[:, :, :].rearrange("p c f -> p (c f)"),
                            po[:, :], xr[:, :, :].rearrange("p c f -> p (c f)"))
                        nc.sync.dma_start(
                            out=out[b, c0:c0 + 3, :, :].rearrange(
                                "c h w -> h c w"),
                            in_=ot[:, :, :])

                groups = [[g0 + i for i in range(2) if g0 + i < BS // 3]
                          for g0 in range(0, BS // 3, 2)]
                previ = None
                for tl in groups:
                    curi = idft_stageA(tl)
                    if previ is not None:
                        idft_stageB(previ)
                    previ = curi
                idft_stageB(previ)

    nc.finalize()
    return nc


_BUILT = None


def _get_built():
    global _BUILT
    if _BUILT is None:
        _BUILT = _build()
    return _BUILT


def _make_in_maps(x, w1, b1, w2, b2):
    fw, s2c, s2s, s2sn, fic, fis, ident = _twiddles()
    in_maps = []
    for k in range(NBLK):
        xs = x[:, k * BS:(k + 1) * BS]
        w1r, w1i = w1[0, k], w1[1, k]
        w2r, w2i = w2[0, k], w2[1, k]
        def pad128(a):
            o = np.zeros((a.shape[0], 128), np.float32)
            o[:, 0:BS] = a
            return o.astype(BF16)
        m = {
            "xt": np.ascontiguousarray(xs.transpose(0, 3, 1, 2)).astype(BF16),
            "xres": np.ascontiguousarray(xs).astype(np.float32),
            "fw": fw, "s2c": s2c, "s2s": s2s, "s2sn": s2sn,
            "fic": fic, "fis": fis, "ident": ident,
            "w1ra": pad128(np.vstack([w1r, b1[0, k][None, :]])),
            "w1ia": pad128(np.vstack([w1i, b1[1, k][None, :]])),
            "w1r": pad128(w1r), "w1in": pad128(-w1i),
            "w2ra": pad128(np.vstack([w2r, b2[0, k][None, :]])),
            "w2ia": pad128(np.vstack([w2i, b2[1, k][None, :]])),
            "w2r": pad128(w2r), "w2in": pad128(-w2i),
        }
        in_maps.append(m)
    return in_maps


def kernel(x, w1, b1, w2, b2):
    from concourse.bass_utils import run_bass_kernel_spmd

    nc = _get_built()
    in_maps = _make_in_maps(x, w1, b1, w2, b2)

    trace = bool(int(os.environ.get("AFNO_TRACE", "0")))
    kw = {}
    if trace:
        import tempfile
        kw["tmpdir"] = tempfile.mkdtemp(prefix="afno_trace_")
        LAST_RESULT["trace_dir"] = kw["tmpdir"]
    res = run_bass_kernel_spmd(nc, in_maps, core_ids=list(range(NBLK)),
                               trace=trace, **kw)
    LAST_RESULT["exec_time_ns"] = res.exec_time_ns

    outp = np.empty((B, C, H, W), np.float32)
    for k in range(NBLK):
        outp[:, k * BS:(k + 1) * BS] = res.results[k]["out"]
    return outp



# revision 2
# speedup vs baseline: 1.0128x; 1.0128x over previous
"""AFNO2D (channel-first) Trainium2 kernel, v2.

out = x + irfft2( softshrink(mlp2(leaky(mlp1(rfft2(x))))) * rfft2(x) )
with block-diagonal complex MLPs over 8 channel blocks of 96.

Sharding: block-parallel - core k owns spectral block k (96 channels), zero
collectives.

v2 design vs baseline: every DFT is a dense bf16 matmul, but the kernel is
restructured to minimize LDWEIGHTS count (one is emitted per matmul
instruction on this toolchain) and to eliminate ALL PE-transposes:

 - S1  (rfft-W, data-stationary): stationary = x slab [w,h], moving = fw
   [128,130].  out u1 [h, c, wfext].
 - S2  (DFT-H, data-stationary): per wf j: 2 matmuls with 256-col combined
   movings s2cs=[cos|-sin], s2sc=[sin|cos].  out utr [c(+ones), 2, j, hf].
 - uT: DMA-XBAR transpose (SBUF->SBUF, off the PE) of utr into
   utT [hf, 2, j, c] for the gating step.
 - mix1: chunks of 512 cols (4 j-slabs), stationaries [97,128] (FWL), out
   o1 [c+1, 2, 512] via ACT Prelu.
 - mix2T (fused transpose): stationary = o1 j-slabs [97,128], moving =
   w2cat [97,192]=[W2r|W2i ; b2r|b2i] -> psum o2^T [hf, (re|im)c] per j.
   No backT transposes needed.
 - shrink (softshrink) + gate in [hf, .] layout (128 partitions), writing
   ybig [hf, 2, j, c].
 - iDFT-H (data-stationary): per channel: stationary = ybig slabs [hf,65],
   movings mcs=[cos|sin], msc=[-sin|cos] 256 cols -> V [wf, (Vr-h|Vi-h)].
 - irfft-W: stationary = fic/fis64 consts, moving = V 4-channel groups
   [65,512] -> psum out [w, 4c, h]; residual add (bf16 x, same [w,c,h]
   staging as S1) fused into the DVE drain; bf16 DMA out (host upcasts).

PSUM rule: matmul start=True clears has_written for the WHOLE bank, so each
bank gets exactly one start=True (its first matmul); all later matmuls use
start=False (fresh ranges overwrite, accumulation ranges add).

Hardcoded shapes: x [4,768,128,128] f32, w1/w2 [2,8,96,96], b1/b2 [2,8,96].
"""

import os
import numpy as np
import ml_dtypes

B, C, H, W = 4, 768, 128, 128
NBLK, BS = 8, 96          # spectral blocks, channels per core
WF = 65                   # rfft size along W
LAM = 0.01                # softshrink threshold
NS = 0.1                  # leaky relu negative slope

BF16 = ml_dtypes.bfloat16

LAST_RESULT = {}          # diagnostics (exec_time_ns) for the test harness


def _twiddles():
    n = 128
    wv = np.arange(n)[:, None].astype(np.float64)
    jv = np.arange(n)[None, :].astype(np.float64)
    ang = 2.0 * np.pi * wv * jv / n  # [128,128]

    # S1 moving operand [w, 130]: cols 0..64 cos/n ; cols 65..129 -sin/n
    # (imag cols 65 and 129 i.e. wf=0,64 are exactly zero)
    fw = np.zeros((n, 130), np.float64)
    fw[:, :WF] = np.cos(ang[:, :WF]) / n
    fw[:, WF + 1:WF + 64] = -np.sin(ang[:, 1:64]) / n

    c = np.cos(ang)
    s = np.sin(ang)
    s2cs = np.hstack([c, -s])     # S2 moving for re-stationary: [cos | -sin]
    s2sc = np.hstack([s, c])      # S2 moving for im-stationary: [sin | cos]
    mcs = np.hstack([c, s])       # iDFT moving for Yr: [cos | sin]
    msc = np.hstack([-s, c])      # iDFT moving for Yi: [-sin | cos]

    # final irfft_W stationaries
    alpha = np.full((WF, 1), 2.0)
    alpha[0, 0] = 1.0
    alpha[64, 0] = 1.0
    fic = alpha * np.cos(2.0 * np.pi * np.arange(WF)[:, None]
                         * np.arange(n)[None, :] / n) / n   # [65, 128]
    fis64 = np.zeros((64, n), np.float64)                   # row 0 (wf=0) = 0
    fis64[1:64] = -2.0 * np.sin(2.0 * np.pi * np.arange(1, 64)[:, None]
                                * np.arange(n)[None, :] / n) / n

    return (fw.astype(BF16), s2cs.astype(BF16), s2sc.astype(BF16),
            mcs.astype(BF16), msc.astype(BF16),
            fic.astype(BF16), fis64.astype(BF16))


def _build():
    import concourse.mybir as mybir
    import concourse.tile as tile
    import concourse.bass as _bass
    from concourse import bacc

    dt = mybir.dt
    AF = mybir.ActivationFunctionType
    ALU = mybir.AluOpType

    nc = bacc.Bacc("TRN2", target_bir_lowering=False, debug=False)

    xt = nc.declare_dram_parameter("xt", [B, W, BS, H], dt.bfloat16, isOutput=False)
    # out is [b, c, w, h]; host transposes the last two axes back
    out = nc.declare_dram_parameter("out", [B, BS, W, H], dt.bfloat16, isOutput=True)

    fw_d = nc.declare_dram_parameter("fw", [128, 130], dt.bfloat16, isOutput=False)
    s2cs_d = nc.declare_dram_parameter("s2cs", [128, 256], dt.bfloat16, isOutput=False)
    s2sc_d = nc.declare_dram_parameter("s2sc", [128, 256], dt.bfloat16, isOutput=False)
    mcs_d = nc.declare_dram_parameter("mcs", [128, 256], dt.bfloat16, isOutput=False)
    msc_d = nc.declare_dram_parameter("msc", [128, 256], dt.bfloat16, isOutput=False)
    fic_d = nc.declare_dram_parameter("fic", [WF, 128], dt.bfloat16, isOutput=False)
    fis_d = nc.declare_dram_parameter("fis64", [64, 128], dt.bfloat16, isOutput=False)

    wnames = ("w1ra", "w1ia", "w1in", "w1r_")
    wshapes = {"w1ra": [BS + 1, 128], "w1ia": [BS + 1, 128],
               "w1in": [BS, 128], "w1r_": [BS, 128],
               "w2cat1": [BS + 1, 192], "w2cat2": [BS, 192]}
    wds = {nm: nc.declare_dram_parameter(nm, wshapes[nm], dt.bfloat16,
                                         isOutput=False)
           for nm in list(wnames) + ["w2cat1", "w2cat2"]}

    dbg = bool(int(os.environ.get("AFNO_DEBUG", "0")))
    if dbg:
        d_u1 = nc.declare_dram_parameter("d_u1", [128, BS, 130], dt.bfloat16, isOutput=True)
        d_utr = nc.declare_dram_parameter("d_utr", [BS + 1, 2, WF, 128], dt.bfloat16, isOutput=True)
        d_utT = nc.declare_dram_parameter("d_utT", [128, 2, WF, BS], dt.bfloat16, isOutput=True)
        d_o1 = nc.declare_dram_parameter("d_o1", [BS + 1, 2, 512], dt.bfloat16, isOutput=True)
        d_ybig = nc.declare_dram_parameter("d_ybig", [128, 2, WF, BS], dt.bfloat16, isOutput=True)
        d_vst = nc.declare_dram_parameter("d_vst", [WF, 4, 256], dt.bfloat16, isOutput=True)

    with tile.TileContext(nc) as tc:
        with (
            tc.tile_pool(name="consts", bufs=1) as consts,
            tc.tile_pool(name="xts", bufs=3) as xts_p,
            tc.tile_pool(name="u1", bufs=1) as u1_p,
            tc.tile_pool(name="utr", bufs=1) as utr_p,
            tc.tile_pool(name="utT", bufs=1) as utT_p,
            tc.tile_pool(name="ybig", bufs=1) as ybig_p,
            tc.tile_pool(name="o1", bufs=3) as o1_p,
            tc.tile_pool(name="sg", bufs=3) as sg_p,
            tc.tile_pool(name="vst", bufs=2) as vst_p,
            tc.tile_pool(name="ot", bufs=3) as ot_p,
            tc.tile_pool(name="pS", bufs=2, space="PSUM") as pS,
            tc.tile_pool(name="pM", bufs=2, space="PSUM") as pM,
            tc.tile_pool(name="pN", bufs=2, space="PSUM") as pN,
        ):
            # ---------------- constants ----------------
            def cload(dparam, shape, tag):
                t = consts.tile(shape, dt.bfloat16, tag=tag, name=tag)
                nc.sync.dma_start(out=t[:], in_=dparam[:, :])
                return t

            fw = cload(fw_d, [128, 130], "fw")
            s2cs = cload(s2cs_d, [128, 256], "s2cs")
            s2sc = cload(s2sc_d, [128, 256], "s2sc")
            mcs = cload(mcs_d, [128, 256], "mcs")
            msc = cload(msc_d, [128, 256], "msc")
            fic = cload(fic_d, [WF, 128], "fic")
            fis64 = cload(fis_d, [64, 128], "fis64")
            wt = {nm: cload(wds[nm], wshapes[nm], nm)
                  for nm in list(wnames) + ["w2cat1", "w2cat2"]}

            for b in range(B):
                # ---------- load x transposed (two halves) ----------
                xh = []
                for hh in range(2):
                    t = xts_p.tile([128, 48, 128], dt.bfloat16, tag="xts", name="xts")
                    nc.sync.dma_start(out=t[:], in_=xt[b, :, hh * 48:(hh + 1) * 48, :])
                    xh.append(t)

                # ---------- S1: rfft along W (data-stationary) ----------
                u1 = u1_p.tile([128, BS, 130], dt.bfloat16, tag="u1")
                for g in range(BS // 3):
                    ps = pS.tile([128, 3, 130], dt.float32, tag="pS", name="ps1")
                    for k in range(3):
                        c = 3 * g + k
                        lhs = xh[c // 48][:, c % 48, :]
                        nc.tensor.matmul(ps[:, k, :], lhs, fw,
                                         start=(k == 0), stop=(k == 2),
                                         skip_group_check=True)
                    nc.any.tensor_copy(u1[:, 3 * g:3 * g + 3, :], ps[:, :, :])
                if dbg and b == 0:
                    nc.sync.dma_start(out=d_u1[:, :, :], in_=u1[:, :, :])

                # ---------- S2: DFT along H, data-stationary per frequency ----
                # utr [97, 2, 65, 128]; row 96 of the re-half is the ones-row
                # for the mix1 bias fold.
                utr = utr_p.tile([BS + 1, 2, WF, 128], dt.bfloat16, tag="utr")
                nc.gpsimd.memset(utr[BS:BS + 1, 0, :, :], 1.0)
                for jp in range(33):          # pairs of j (last pair = j=64 alone)
                    js = [j for j in (2 * jp, 2 * jp + 1) if j < WF]
                    ps = pS.tile([128, 2, 256], dt.float32, tag="pS", name="ps2")
                    for q, j in enumerate(js):
                        lr = u1[:, :, j]
                        li = u1[:, :, 65 + j]
                        nc.tensor.matmul(ps[0:BS, q, :], lr, s2cs,
                                         start=(q == 0), stop=False,
                                         skip_group_check=True)
                        nc.tensor.matmul(ps[0:BS, q, :], li, s2sc,
                                         start=False, stop=(q == len(js) - 1),
                                         skip_group_check=True)
                    src = ps[0:BS, 0:len(js), :].rearrange(
                        "p q (r h) -> p q r h", r=2)
                    dst = utr[0:BS, :, js[0]:js[0] + len(js), :].rearrange(
                        "p r j h -> p j r h")
                    nc.any.tensor_copy(dst, src)
                if dbg and b == 0:
                    nc.sync.dma_start(out=d_utr[:, :, :, :], in_=utr[:, :, :, :])

                # ---------- uT: XBAR transpose utr -> utT [hf, 2, j, c] ------
                utT = utT_p.tile([128, 2, WF, BS], dt.bfloat16, tag="utT")
                for ri in range(2):
                    for j0 in range(0, WF, 4):
                        njj = min(4, WF - j0)
                        nc.sync.dma_start_transpose(
                            out=utT[:, ri, j0:j0 + njj, :],
                            in_=utr[0:BS, ri, j0:j0 + njj, :])
                if dbg and b == 0:
                    nc.sync.dma_start(out=d_utT[:, :, :, :], in_=utT[:, :, :, :])

                # ---------- mix: chunks of 512 (4 j-slabs) -------------------
                # software pipeline: emit mix1(c+1) before mix2T/shrink/gate(c)
                ybig = ybig_p.tile([128, 2, WF, BS], dt.bfloat16, tag="ybig")
                nchunk = 17                   # 16 x 512 + 1 x 128

                def mix1(ci):
                    j0 = 4 * ci
                    njj = min(4, WF - j0)
                    sz = njj * 128
                    ur = utr[0:BS + 1, 0, j0:j0 + njj, :]
                    ui = utr[0:BS, 1, j0:j0 + njj, :]
                    p1 = pM.tile([128, 1024], dt.float32, tag="pM", name="p1")
                    nc.tensor.matmul(p1[:, 0:sz], wt["w1ra"], ur,
                                     start=True, stop=False, skip_group_check=True)
                    nc.tensor.matmul(p1[:, 512:512 + sz], wt["w1ia"], ur,
                                     start=True, stop=False, skip_group_check=True)
                    nc.tensor.matmul(p1[:, 0:sz], wt["w1in"], ui,
                                     start=False, stop=False, skip_group_check=True)
                    nc.tensor.matmul(p1[:, 512:512 + sz], wt["w1r_"], ui,
                                     start=False, stop=True, skip_group_check=True)
                    o1 = o1_p.tile([BS + 1, 2, 512], dt.bfloat16, tag="o1",
                                   name="o1")
                    nc.gpsimd.memset(o1[BS:BS + 1, 0, 0:sz], 1.0)
                    src = p1[0:BS, :].rearrange("p (r f) -> p r f", r=2)[:, :, 0:sz]
                    nc.scalar.activation(o1[0:BS, :, 0:sz], src, AF.Prelu,
                                         bias=0.0, scale=1.0, alpha=NS)
                    return (ci, j0, njj, o1)

                def mix2T(st):
                    ci, j0, njj, o1 = st
                    if dbg and b == 0 and ci == 0:
                        nc.sync.dma_start(out=d_o1[:, :, :], in_=o1[:, :, :])
                    for t0 in range(0, njj, 2):
                        js = [jj for jj in (t0, t0 + 1) if jj < njj]
                        p2 = pN.tile([128, 2, 192], dt.float32, tag="pN",
                                     name="p2")
                        for q, jj in enumerate(js):
                            o1r = o1[0:BS + 1, 0, 128 * jj:128 * jj + 128]
                            o1i = o1[0:BS, 1, 128 * jj:128 * jj + 128]
                            nc.tensor.matmul(p2[:, q, :], o1r, wt["w2cat1"],
                                             start=(q == 0), stop=False,
                                             skip_group_check=True)
                            nc.tensor.matmul(p2[:, q, :], o1i, wt["w2cat2"],
                                             start=False, stop=(q == len(js) - 1),
                                             skip_group_check=True)
                        # softshrink: s = p2 - clamp(p2, +-LAM)   [128, 2, 192]
                        nq = len(js)
                        cl = sg_p.tile([128, 2, 192], dt.bfloat16, tag="cl",
                                       name="cl")
                        nc.vector.tensor_scalar(cl[:, 0:nq, :], p2[:, 0:nq, :],
                                                -LAM, LAM, ALU.max, ALU.min)
                        sh = sg_p.tile([128, 2, 192], dt.bfloat16, tag="sh",
                                       name="sh")
                        nc.vector.tensor_sub(sh[:, 0:nq, :], p2[:, 0:nq, :],
                                             cl[:, 0:nq, :])
                        # gate: y = s * u (complex), in [hf, c] layout
                        jg0 = j0 + t0
                        _s = sh[:, :, :]
                        sr2 = _bass.AP(
                            tensor=_s.tensor, offset=_s.offset,
                            ap=[_s.ap[0], [192, nq], [0, 2], [1, BS]])
                        si2 = _bass.AP(
                            tensor=_s.tensor, offset=_s.offset + 96,
                            ap=[_s.ap[0], [192, nq], [0, 2], [1, BS]])
                        u2 = utT[:, :, jg0:jg0 + nq, :].rearrange(
                            "p r j c -> p j r c")
                        ta = sg_p.tile([128, 2, 2, BS], dt.bfloat16, tag="ta",
                                       name="ta")
                        tb = sg_p.tile([128, 2, 2, BS], dt.bfloat16, tag="tb",
                                       name="tb")
                        nc.vector.tensor_mul(ta[:, 0:nq, :, :], sr2, u2)
                        nc.gpsimd.tensor_mul(tb[:, 0:nq, :, :], si2, u2)
                        nc.vector.tensor_sub(
                            ybig[:, 0, jg0:jg0 + nq, :],
                            ta[:, 0:nq, 0, :], tb[:, 0:nq, 1, :])
                        nc.gpsimd.tensor_add(
                            ybig[:, 1, jg0:jg0 + nq, :],
                            ta[:, 0:nq, 1, :], tb[:, 0:nq, 0, :])

                prev = None
                for ci in range(nchunk):
                    cur = mix1(ci)
                    if prev is not None:
                        mix2T(prev)
                    prev = cur
                mix2T(prev)
                if dbg and b == 0:
                    nc.sync.dma_start(out=d_ybig[:, :, :, :], in_=ybig[:, :, :, :])

                # ---------- iDFT-H (per channel pair) + irfft-W + out --------
                def idftA(p):
                    # pair p: channels 2p, 2p+1 -> pV [65, 2, (Vr-h | Vi-h)]
                    pV = pN.tile([128, 2, 256], dt.float32, tag="pN", name="pV")
                    for cc in range(2):
                        c = 2 * p + cc
                        yr = ybig[:, 0, :, c]
                        yi = ybig[:, 1, :, c]
                        nc.tensor.matmul(pV[0:WF, cc, :], yr, mcs,
                                         start=(cc == 0), stop=False,
                                         skip_group_check=True)
                        nc.tensor.matmul(pV[0:WF, cc, :], yi, msc,
                                         start=False, stop=(cc == 1),
                                         skip_group_check=True)
                    return pV

                def vdrain(p, pV, vst):
                    q = p % 2
                    nc.any.tensor_copy(vst[0:WF, 2 * q:2 * q + 2, :],
                                       pV[0:WF, :, :])

                def final(qd, vst):
                    # quad qd: channels 4qd..4qd+3
                    pO = pM.tile([128, 4, 128], dt.float32, tag="pM", name="pO")
                    nc.tensor.matmul(pO[:, :, :], fic, vst[0:WF, :, 0:128],
                                     start=True, stop=False,
                                     skip_group_check=True)
                    nc.tensor.matmul(pO[:, :, :], fis64, vst[0:64, :, 128:256],
                                     start=False, stop=True,
                                     skip_group_check=True)
                    c0 = 4 * qd
                    hh = c0 // 48
                    xh4 = xh[hh][:, c0 % 48:c0 % 48 + 4, :]
                    ot = ot_p.tile([128, 4, 128], dt.bfloat16, tag="ot",
                                   name="ot")
                    nc.vector.tensor_add(
                        ot[:, :, :].rearrange("p c f -> p (c f)"),
                        pO[:, :, :].rearrange("p c f -> p (c f)"),
                        xh4.rearrange("p c f -> p (c f)"))
                    nc.sync.dma_start(
                        out=out[b, c0:c0 + 4, :, :].rearrange("c w h -> w c h"),
                        in_=ot[:, :, :])

                # pipeline: idft(p) two ahead of final(quad)
                vsts = {}
                pvq = []
                for p in range(BS // 2):
                    qd = p // 2
                    if p % 2 == 0:
                        vsts[qd] = vst_p.tile([WF, 4, 256], dt.bfloat16,
                                              tag="vst", name="vst")
                    pV = idftA(p)
                    vdrain(p, pV, vsts[qd])
                    if dbg and b == 0 and p == 1:
                        nc.sync.dma_start(out=d_vst[:, :, :], in_=vsts[0][:, :, :])
                    if p % 2 == 1 and p >= 3:
                        final(qd - 1, vsts[qd - 1])
                final(BS // 4 - 1, vsts[BS // 4 - 1])

    nc.finalize()
    return nc


_BUILT = None


def _get_built():
    global _BUILT
    if _BUILT is None:
        _BUILT = _build()
    return _BUILT


def _make_in_maps(x, w1, b1, w2, b2):
    fw, s2cs, s2sc, mcs, msc, fic, fis64 = _twiddles()
    in_maps = []
    for k in range(NBLK):
        xs = x[:, k * BS:(k + 1) * BS]
        w1r, w1i = w1[0, k], w1[1, k]
        w2r, w2i = w2[0, k], w2[1, k]

        def pad128(a):
            o = np.zeros((a.shape[0], 128), np.float32)
            o[:, 0:BS] = a
            return o.astype(BF16)

        w2cat1 = np.vstack([np.hstack([w2r, w2i]),
                            np.hstack([b2[0, k][None, :], b2[1, k][None, :]])])
        w2cat2 = np.hstack([-w2i, w2r])
        m = {
            "xt": np.ascontiguousarray(xs.transpose(0, 3, 1, 2)).astype(BF16),
            "fw": fw, "s2cs": s2cs, "s2sc": s2sc, "mcs": mcs, "msc": msc,
            "fic": fic, "fis64": fis64,
            "w1ra": pad128(np.vstack([w1r, b1[0, k][None, :]])),
            "w1ia": pad128(np.vstack([w1i, b1[1, k][None, :]])),
            "w1in": pad128(-w1i), "w1r_": pad128(w1r),
            "w2cat1": w2cat1.astype(BF16), "w2cat2": w2cat2.astype(BF16),
        }
        in_maps.append(m)
    return in_maps


def kernel(x, w1, b1, w2, b2):
    from concourse.bass_utils import run_bass_kernel_spmd

    nc = _get_built()
    in_maps = _make_in_maps(x, w1, b1, w2, b2)

    trace = bool(int(os.environ.get("AFNO_TRACE", "0")))
    kw = {}
    if trace:
        import tempfile
        kw["tmpdir"] = tempfile.mkdtemp(prefix="afno_trace_")
        LAST_RESULT["trace_dir"] = kw["tmpdir"]
    res = run_bass_kernel_spmd(nc, in_maps, core_ids=list(range(NBLK)),
                               trace=trace, **kw)
    LAST_RESULT["exec_time_ns"] = res.exec_time_ns
    LAST_RESULT["results"] = res.results

    outp = np.empty((B, C, H, W), np.float32)
    for k in range(NBLK):
        outp[:, k * BS:(k + 1) * BS] = \
            res.results[k]["out"].astype(np.float32).transpose(0, 1, 3, 2)
    return outp


# revision 3
# speedup vs baseline: 1.0136x; 1.0008x over previous
"""AFNO2D (channel-first) Trainium2 kernel, v2.

out = x + irfft2( softshrink(mlp2(leaky(mlp1(rfft2(x))))) * rfft2(x) )
with block-diagonal complex MLPs over 8 channel blocks of 96.

Sharding: block-parallel - core k owns spectral block k (96 channels), zero
collectives.

v2 design vs baseline: every DFT is a dense bf16 matmul, but the kernel is
restructured to minimize LDWEIGHTS count (one is emitted per matmul
instruction on this toolchain) and to eliminate ALL PE-transposes:

 - S1  (rfft-W, data-stationary): stationary = x slab [w,h], moving = fw
   [128,130].  out u1 [h, c, wfext].
 - S2  (DFT-H, data-stationary): per wf j: 2 matmuls with 256-col combined
   movings s2cs=[cos|-sin], s2sc=[sin|cos].  out utr [c(+ones), 2, j, hf].
 - uT: DMA-XBAR transpose (SBUF->SBUF, off the PE) of utr into
   utT [hf, 2, j, c] for the gating step.
 - mix1: chunks of 512 cols (4 j-slabs), stationaries [97,128] (FWL), out
   o1 [c+1, 2, 512] via ACT Prelu.
 - mix2T (fused transpose): stationary = o1 j-slabs [97,128], moving =
   w2cat [97,192]=[W2r|W2i ; b2r|b2i] -> psum o2^T [hf, (re|im)c] per j.
   No backT transposes needed.
 - shrink (softshrink) + gate in [hf, .] layout (128 partitions), writing
   ybig [hf, 2, j, c].
 - iDFT-H (data-stationary): per channel: stationary = ybig slabs [hf,65],
   movings mcs=[cos|sin], msc=[-sin|cos] 256 cols -> V [wf, (Vr-h|Vi-h)].
 - irfft-W: stationary = fic/fis64 consts, moving = V 4-channel groups
   [65,512] -> psum out [w, 4c, h]; residual add (bf16 x, same [w,c,h]
   staging as S1) fused into the DVE drain; bf16 DMA out (host upcasts).

PSUM rule: matmul start=True clears has_written for the WHOLE bank, so each
bank gets exactly one start=True (its first matmul); all later matmuls use
start=False (fresh ranges overwrite, accumulation ranges add).

Hardcoded shapes: x [4,768,128,128] f32, w1/w2 [2,8,96,96], b1/b2 [2,8,96].
"""

import os
import numpy as np
import ml_dtypes

B, C, H, W = 4, 768, 128, 128
NBLK, BS = 8, 96          # spectral blocks, channels per core
WF = 65                   # rfft size along W
LAM = 0.01                # softshrink threshold
NS = 0.1                  # leaky relu negative slope

BF16 = ml_dtypes.bfloat16

LAST_RESULT = {}          # diagnostics (exec_time_ns) for the test harness


def _twiddles():
    n = 128
    wv = np.arange(n)[:, None].astype(np.float64)
    jv = np.arange(n)[None, :].astype(np.float64)
    ang = 2.0 * np.pi * wv * jv / n  # [128,128]

    # S1 moving operand [w, 130]: cols 0..64 cos/n ; cols 65..129 -sin/n
    # (imag cols 65 and 129 i.e. wf=0,64 are exactly zero)
    fw = np.zeros((n, 130), np.float64)
    fw[:, :WF] = np.cos(ang[:, :WF]) / n
    fw[:, WF + 1:WF + 64] = -np.sin(ang[:, 1:64]) / n

    c = np.cos(ang)
    s = np.sin(ang)
    s2cs = np.hstack([c, -s])     # S2 moving for re-stationary: [cos | -sin]
    s2sc = np.hstack([s, c])      # S2 moving for im-stationary: [sin | cos]
    mcs = np.hstack([c, s])       # iDFT moving for Yr: [cos | sin]
    msc = np.hstack([-s, c])      # iDFT moving for Yi: [-sin | cos]

    # final irfft_W stationaries
    alpha = np.full((WF, 1), 2.0)
    alpha[0, 0] = 1.0
    alpha[64, 0] = 1.0
    fic = alpha * np.cos(2.0 * np.pi * np.arange(WF)[:, None]
                         * np.arange(n)[None, :] / n) / n   # [65, 128]
    fis64 = np.zeros((64, n), np.float64)                   # row 0 (wf=0) = 0
    fis64[1:64] = -2.0 * np.sin(2.0 * np.pi * np.arange(1, 64)[:, None]
                                * np.arange(n)[None, :] / n) / n

    return (fw.astype(BF16), s2cs.astype(BF16), s2sc.astype(BF16),
            mcs.astype(BF16), msc.astype(BF16),
            fic.astype(BF16), fis64.astype(BF16))


def _build():
    import concourse.mybir as mybir
    import concourse.tile as tile
    import concourse.bass as _bass
    from concourse import bacc

    dt = mybir.dt
    AF = mybir.ActivationFunctionType
    ALU = mybir.AluOpType

    nc = bacc.Bacc("TRN2", target_bir_lowering=False, debug=False)

    xt = nc.declare_dram_parameter("xt", [B, W, BS, H], dt.bfloat16, isOutput=False)
    # out is [b, c, w, h]; host transposes the last two axes back
    out = nc.declare_dram_parameter("out", [B, BS, W, H], dt.bfloat16, isOutput=True)

    fw_d = nc.declare_dram_parameter("fw", [128, 130], dt.bfloat16, isOutput=False)
    s2cs_d = nc.declare_dram_parameter("s2cs", [128, 256], dt.bfloat16, isOutput=False)
    s2sc_d = nc.declare_dram_parameter("s2sc", [128, 256], dt.bfloat16, isOutput=False)
    mcs_d = nc.declare_dram_parameter("mcs", [128, 256], dt.bfloat16, isOutput=False)
    msc_d = nc.declare_dram_parameter("msc", [128, 256], dt.bfloat16, isOutput=False)
    fic_d = nc.declare_dram_parameter("fic", [WF, 128], dt.bfloat16, isOutput=False)
    fis_d = nc.declare_dram_parameter("fis64", [64, 128], dt.bfloat16, isOutput=False)

    wnames = ("w1ra", "w1ia", "w1in", "w1r_")
    wshapes = {"w1ra": [BS + 1, 128], "w1ia": [BS + 1, 128],
               "w1in": [BS, 128], "w1r_": [BS, 128],
               "w2cat1": [BS + 1, 192], "w2cat2": [BS, 192]}
    wds = {nm: nc.declare_dram_parameter(nm, wshapes[nm], dt.bfloat16,
                                         isOutput=False)
           for nm in list(wnames) + ["w2cat1", "w2cat2"]}

    dbg = bool(int(os.environ.get("AFNO_DEBUG", "0")))
    if dbg:
        d_u1 = nc.declare_dram_parameter("d_u1", [128, BS, 130], dt.bfloat16, isOutput=True)
        d_utr = nc.declare_dram_parameter("d_utr", [BS + 1, 2, WF, 128], dt.bfloat16, isOutput=True)
        d_utT = nc.declare_dram_parameter("d_utT", [128, 2, WF, BS], dt.bfloat16, isOutput=True)
        d_o1 = nc.declare_dram_parameter("d_o1", [BS + 1, 2, 512], dt.bfloat16, isOutput=True)
        d_ybig = nc.declare_dram_parameter("d_ybig", [128, 2, WF, BS], dt.bfloat16, isOutput=True)
        d_vst = nc.declare_dram_parameter("d_vst", [WF, 4, 256], dt.bfloat16, isOutput=True)

    with tile.TileContext(nc) as tc:
        with (
            tc.tile_pool(name="consts", bufs=1) as consts,
            tc.tile_pool(name="xts", bufs=4) as xts_p,
            tc.tile_pool(name="u1", bufs=1) as u1_p,
            tc.tile_pool(name="utr", bufs=1) as utr_p,
            tc.tile_pool(name="utT", bufs=1) as utT_p,
            tc.tile_pool(name="ybig", bufs=1) as ybig_p,
            tc.tile_pool(name="o1", bufs=3) as o1_p,
            tc.tile_pool(name="sg", bufs=3) as sg_p,
            tc.tile_pool(name="vst", bufs=2) as vst_p,
            tc.tile_pool(name="ot", bufs=3) as ot_p,
            tc.tile_pool(name="pS", bufs=2, space="PSUM") as pS,
            tc.tile_pool(name="pM", bufs=2, space="PSUM") as pM,
            tc.tile_pool(name="pN", bufs=2, space="PSUM") as pN,
        ):
            # ---------------- constants ----------------
            def cload(dparam, shape, tag):
                t = consts.tile(shape, dt.bfloat16, tag=tag, name=tag)
                nc.sync.dma_start(out=t[:], in_=dparam[:, :])
                return t

            fw = cload(fw_d, [128, 130], "fw")
            s2cs = cload(s2cs_d, [128, 256], "s2cs")
            s2sc = cload(s2sc_d, [128, 256], "s2sc")
            mcs = cload(mcs_d, [128, 256], "mcs")
            msc = cload(msc_d, [128, 256], "msc")
            fic = cload(fic_d, [WF, 128], "fic")
            fis64 = cload(fis_d, [64, 128], "fis64")
            wt = {nm: cload(wds[nm], wshapes[nm], nm)
                  for nm in list(wnames) + ["w2cat1", "w2cat2"]}

            def load_x(b):
                xh = []
                for hh in range(2):
                    t = xts_p.tile([128, 48, 128], dt.bfloat16, tag="xts", name="xts")
                    nc.sync.dma_start(out=t[:], in_=xt[b, :, hh * 48:(hh + 1) * 48, :])
                    xh.append(t)
                return xh

            def s1(b, xh):
                u1 = u1_p.tile([128, BS, 130], dt.bfloat16, tag="u1")
                for g in range(BS // 3):
                    ps = pS.tile([128, 3, 130], dt.float32, tag="pS", name="ps1")
                    for k in range(3):
                        c = 3 * g + k
                        lhs = xh[c // 48][:, c % 48, :]
                        nc.tensor.matmul(ps[:, k, :], lhs, fw,
                                         start=(k == 0), stop=(k == 2),
                                         skip_group_check=True)
                    # pinned to ACT: during the mix(b) tail the DVE/GpSimd
                    # FIFOs are full of gate ops; ACT is idle there, so S1(b+1)
                    # psum rotation must not queue behind DVE.
                    nc.scalar.activation(u1[:, 3 * g:3 * g + 3, :], ps[:, :, :],
                                         AF.Copy, bias=0.0, scale=1.0)
                if dbg and b == 0:
                    nc.sync.dma_start(out=d_u1[:, :, :], in_=u1[:, :, :])
                return u1

            def s2(b, u1):
                # DFT along H, data-stationary per frequency; uT XBAR
                # transposes issued in 3 big j-groups interleaved in.
                utr = utr_p.tile([BS + 1, 2, WF, 128], dt.bfloat16, tag="utr")
                utT = utT_p.tile([128, 2, WF, BS], dt.bfloat16, tag="utT")
                nc.gpsimd.memset(utr[BS:BS + 1, 0, :, :], 1.0)
                xbar_after = {11: (0, 24), 23: (24, 48), 32: (48, WF)}
                for jp in range(33):
                    js = [j for j in (2 * jp, 2 * jp + 1) if j < WF]
                    ps = pS.tile([128, 2, 256], dt.float32, tag="pS", name="ps2")
                    for q, j in enumerate(js):
                        lr = u1[:, :, j]
                        li = u1[:, :, 65 + j]
                        nc.tensor.matmul(ps[0:BS, q, :], lr, s2cs,
                                         start=(q == 0), stop=False,
                                         skip_group_check=True)
                        nc.tensor.matmul(ps[0:BS, q, :], li, s2sc,
                                         start=False, stop=(q == len(js) - 1),
                                         skip_group_check=True)
                    src = ps[0:BS, 0:len(js), :].rearrange(
                        "p q (r h) -> p q r h", r=2)
                    dst = utr[0:BS, :, js[0]:js[0] + len(js), :].rearrange(
                        "p r j h -> p j r h")
                    nc.any.tensor_copy(dst, src)
                    if jp in xbar_after:
                        lo, hi = xbar_after[jp]
                        for ri in range(2):
                            nc.sync.dma_start_transpose(
                                out=utT[:, ri, lo:hi, :],
                                in_=utr[0:BS, ri, lo:hi, :])
                if dbg and b == 0:
                    nc.sync.dma_start(out=d_utr[:, :, :, :], in_=utr[:, :, :, :])
                return utr, utT

            def mix(b, utr, utT):
                ybig = ybig_p.tile([128, 2, WF, BS], dt.bfloat16, tag="ybig")
                nchunk = 17                   # 16 x 512 + 1 x 128

                def mix1(ci):
                    j0 = 4 * ci
                    njj = min(4, WF - j0)
                    sz = njj * 128
                    ur = utr[0:BS + 1, 0, j0:j0 + njj, :]
                    ui = utr[0:BS, 1, j0:j0 + njj, :]
                    p1 = pM.tile([128, 1024], dt.float32, tag="pM", name="p1")
                    nc.tensor.matmul(p1[:, 0:sz], wt["w1ra"], ur,
                                     start=True, stop=False, skip_group_check=True)
                    nc.tensor.matmul(p1[:, 512:512 + sz], wt["w1ia"], ur,
                                     start=True, stop=False, skip_group_check=True)
                    nc.tensor.matmul(p1[:, 0:sz], wt["w1in"], ui,
                                     start=False, stop=False, skip_group_check=True)
                    nc.tensor.matmul(p1[:, 512:512 + sz], wt["w1r_"], ui,
                                     start=False, stop=True, skip_group_check=True)
                    o1 = o1_p.tile([BS + 1, 2, 512], dt.bfloat16, tag="o1",
                                   name="o1")
                    nc.any.tensor_copy(
                        o1[BS:BS + 1, 0, 0:sz],
                        utr[BS:BS + 1, 0, j0:j0 + njj, :])
                    psrc = p1[0:BS, :].rearrange("p (r f) -> p r f", r=2)[:, :, 0:sz]
                    nc.scalar.activation(o1[0:BS, :, 0:sz], psrc, AF.Prelu,
                                         bias=0.0, scale=1.0, alpha=NS)
                    return (ci, j0, njj, o1)

                def mix2T(st):
                    ci, j0, njj, o1 = st
                    if dbg and b == 0 and ci == 0:
                        nc.sync.dma_start(out=d_o1[:, :, :], in_=o1[:, :, :])
                    for t0 in range(0, njj, 2):
                        js = [jj for jj in (t0, t0 + 1) if jj < njj]
                        p2 = pN.tile([128, 2, 192], dt.float32, tag="pN",
                                     name="p2")
                        for q, jj in enumerate(js):
                            o1r = o1[0:BS + 1, 0, 128 * jj:128 * jj + 128]
                            o1i = o1[0:BS, 1, 128 * jj:128 * jj + 128]
                            nc.tensor.matmul(p2[:, q, :], o1r, wt["w2cat1"],
                                             start=(q == 0), stop=False,
                                             skip_group_check=True)
                            nc.tensor.matmul(p2[:, q, :], o1i, wt["w2cat2"],
                                             start=False, stop=(q == len(js) - 1),
                                             skip_group_check=True)
                        # softshrink: s = p2 - clamp(p2, +-LAM)   [128, 2, 192]
                        nq = len(js)
                        cl = sg_p.tile([128, 2, 192], dt.bfloat16, tag="cl",
                                       name="cl")
                        nc.vector.tensor_scalar(cl[:, 0:nq, :], p2[:, 0:nq, :],
                                                -LAM, LAM, ALU.max, ALU.min)
                        sh = sg_p.tile([128, 2, 192], dt.bfloat16, tag="sh",
                                       name="sh")
                        nc.vector.tensor_sub(sh[:, 0:nq, :], p2[:, 0:nq, :],
                                             cl[:, 0:nq, :])
                        # gate: y = s * u (complex), in [hf, c] layout
                        jg0 = j0 + t0
                        _s = sh[:, :, :]
                        sr2 = _bass.AP(
                            tensor=_s.tensor, offset=_s.offset,
                            ap=[_s.ap[0], [192, nq], [0, 2], [1, BS]])
                        si2 = _bass.AP(
                            tensor=_s.tensor, offset=_s.offset + 96,
                            ap=[_s.ap[0], [192, nq], [0, 2], [1, BS]])
                        u2 = utT[:, :, jg0:jg0 + nq, :].rearrange(
                            "p r j c -> p j r c")
                        ta = sg_p.tile([128, 2, 2, BS], dt.bfloat16, tag="ta",
                                       name="ta")
                        tb = sg_p.tile([128, 2, 2, BS], dt.bfloat16, tag="tb",
                                       name="tb")
                        nc.vector.tensor_mul(ta[:, 0:nq, :, :], sr2, u2)
                        nc.gpsimd.tensor_mul(tb[:, 0:nq, :, :], si2, u2)
                        nc.vector.tensor_sub(
                            ybig[:, 0, jg0:jg0 + nq, :],
                            ta[:, 0:nq, 0, :], tb[:, 0:nq, 1, :])
                        nc.gpsimd.tensor_add(
                            ybig[:, 1, jg0:jg0 + nq, :],
                            ta[:, 0:nq, 1, :], tb[:, 0:nq, 0, :])

                prev = None
                for ci in range(nchunk):
                    cur = mix1(ci)
                    if prev is not None:
                        mix2T(prev)
                    prev = cur
                mix2T(prev)
                if dbg and b == 0:
                    nc.sync.dma_start(out=d_ybig[:, :, :, :], in_=ybig[:, :, :, :])
                return ybig

            def idft_final(b, ybig, xh):
                def idftA(p):
                    pV = pN.tile([128, 2, 256], dt.float32, tag="pN", name="pV")
                    for cc in range(2):
                        c = 2 * p + cc
                        yr = ybig[:, 0, :, c]
                        yi = ybig[:, 1, :, c]
                        nc.tensor.matmul(pV[0:WF, cc, :], yr, mcs,
                                         start=(cc == 0), stop=False,
                                         skip_group_check=True)
                        nc.tensor.matmul(pV[0:WF, cc, :], yi, msc,
                                         start=False, stop=(cc == 1),
                                         skip_group_check=True)
                    return pV

                def vdrain(p, pV, vst):
                    q = p % 2
                    nc.any.tensor_copy(vst[0:WF, 2 * q:2 * q + 2, :],
                                       pV[0:WF, :, :])

                def final(qd, vst):
                    pO = pM.tile([128, 4, 128], dt.float32, tag="pM", name="pO")
                    nc.tensor.matmul(pO[:, :, :], fic, vst[0:WF, :, 0:128],
                                     start=True, stop=False,
                                     skip_group_check=True)
                    nc.tensor.matmul(pO[:, :, :], fis64, vst[0:64, :, 128:256],
                                     start=False, stop=True,
                                     skip_group_check=True)
                    c0 = 4 * qd
                    hh = c0 // 48
                    xh4 = xh[hh][:, c0 % 48:c0 % 48 + 4, :]
                    ot = ot_p.tile([128, 4, 128], dt.bfloat16, tag="ot",
                                   name="ot")
                    nc.vector.tensor_add(
                        ot[:, :, :].rearrange("p c f -> p (c f)"),
                        pO[:, :, :].rearrange("p c f -> p (c f)"),
                        xh4.rearrange("p c f -> p (c f)"))
                    nc.sync.dma_start(
                        out=out[b, c0:c0 + 4, :, :].rearrange("c w h -> w c h"),
                        in_=ot[:, :, :])

                vsts = {}
                for p in range(BS // 2):
                    qd = p // 2
                    if p % 2 == 0:
                        vsts[qd] = vst_p.tile([WF, 4, 256], dt.bfloat16,
                                              tag="vst", name="vst")
                    pV = idftA(p)
                    vdrain(p, pV, vsts[qd])
                    if dbg and b == 0 and p == 1:
                        nc.sync.dma_start(out=d_vst[:, :, :], in_=vsts[0][:, :, :])
                    if p % 2 == 1 and p >= 3:
                        final(qd - 1, vsts[qd - 1])
                final(BS // 4 - 1, vsts[BS // 4 - 1])

            # ---- batch-level software pipeline: S1(b+1) fills the PE while
            # ---- the mix(b) elementwise tail drains; S2(b+1) follows idft(b).
            xh_all = {0: load_x(0)}
            u1 = s1(0, xh_all[0])
            utr, utT = s2(0, u1)
            for b in range(B):
                if b + 1 < B:
                    xh_all[b + 1] = load_x(b + 1)
                ybig = mix(b, utr, utT)
                if b + 1 < B:
                    u1 = s1(b + 1, xh_all[b + 1])
                idft_final(b, ybig, xh_all[b])
                del xh_all[b]
                if b + 1 < B:
                    utr, utT = s2(b + 1, u1)

    nc.finalize()
    return nc


_BUILT = None


def _get_built():
    global _BUILT
    if _BUILT is None:
        _BUILT = _build()
    return _BUILT


def _make_in_maps(x, w1, b1, w2, b2):
    fw, s2cs, s2sc, mcs, msc, fic, fis64 = _twiddles()
    in_maps = []
    for k in range(NBLK):
        xs = x[:, k * BS:(k + 1) * BS]
        w1r, w1i = w1[0, k], w1[1, k]
        w2r, w2i = w2[0, k], w2[1, k]

        def pad128(a):
            o = np.zeros((a.shape[0], 128), np.float32)
            o[:, 0:BS] = a
            return o.astype(BF16)

        w2cat1 = np.vstack([np.hstack([w2r, w2i]),
                            np.hstack([b2[0, k][None, :], b2[1, k][None, :]])])
        w2cat2 = np.hstack([-w2i, w2r])
        m = {
            "xt": np.ascontiguousarray(xs.transpose(0, 3, 1, 2)).astype(BF16),
            "fw": fw, "s2cs": s2cs, "s2sc": s2sc, "mcs": mcs, "msc": msc,
            "fic": fic, "fis64": fis64,
            "w1ra": pad128(np.vstack([w1r, b1[0, k][None, :]])),
            "w1ia": pad128(np.vstack([w1i, b1[1, k][None, :]])),
            "w1in": pad128(-w1i), "w1r_": pad128(w1r),
            "w2cat1": w2cat1.astype(BF16), "w2cat2": w2cat2.astype(BF16),
        }
        in_maps.append(m)
    return in_maps


def kernel(x, w1, b1, w2, b2):
    from concourse.bass_utils import run_bass_kernel_spmd

    nc = _get_built()
    in_maps = _make_in_maps(x, w1, b1, w2, b2)

    trace = bool(int(os.environ.get("AFNO_TRACE", "0")))
    kw = {}
    if trace:
        import tempfile
        kw["tmpdir"] = tempfile.mkdtemp(prefix="afno_trace_")
        LAST_RESULT["trace_dir"] = kw["tmpdir"]
    res = run_bass_kernel_spmd(nc, in_maps, core_ids=list(range(NBLK)),
                               trace=trace, **kw)
    LAST_RESULT["exec_time_ns"] = res.exec_time_ns
    LAST_RESULT["results"] = res.results

    outp = np.empty((B, C, H, W), np.float32)
    for k in range(NBLK):
        outp[:, k * BS:(k + 1) * BS] = \
            res.results[k]["out"].astype(np.float32).transpose(0, 1, 3, 2)
    return outp


# revision 4
# speedup vs baseline: 1.0145x; 1.0009x over previous
"""AFNO2D (channel-first) Trainium2 kernel, v2.

out = x + irfft2( softshrink(mlp2(leaky(mlp1(rfft2(x))))) * rfft2(x) )
with block-diagonal complex MLPs over 8 channel blocks of 96.

Sharding: block-parallel - core k owns spectral block k (96 channels), zero
collectives.

v2 design vs baseline: every DFT is a dense bf16 matmul, but the kernel is
restructured to minimize LDWEIGHTS count (one is emitted per matmul
instruction on this toolchain) and to eliminate ALL PE-transposes:

 - S1  (rfft-W, data-stationary): stationary = x slab [w,h], moving = fw
   [128,130].  out u1 [h, c, wfext].
 - S2  (DFT-H, data-stationary): per wf j: 2 matmuls with 256-col combined
   movings s2cs=[cos|-sin], s2sc=[sin|cos].  out utr [c(+ones), 2, j, hf].
 - uT: DMA-XBAR transpose (SBUF->SBUF, off the PE) of utr into
   utT [hf, 2, j, c] for the gating step.
 - mix1: chunks of 512 cols (4 j-slabs), stationaries [97,128] (FWL), out
   o1 [c+1, 2, 512] via ACT Prelu.
 - mix2T (fused transpose): stationary = o1 j-slabs [97,128], moving =
   w2cat [97,192]=[W2r|W2i ; b2r|b2i] -> psum o2^T [hf, (re|im)c] per j.
   No backT transposes needed.
 - shrink (softshrink) + gate in [hf, .] layout (128 partitions), writing
   ybig [hf, 2, j, c].
 - iDFT-H (data-stationary): per channel: stationary = ybig slabs [hf,65],
   movings mcs=[cos|sin], msc=[-sin|cos] 256 cols -> V [wf, (Vr-h|Vi-h)].
 - irfft-W: stationary = fic/fis64 consts, moving = V 4-channel groups
   [65,512] -> psum out [w, 4c, h]; residual add (bf16 x, same [w,c,h]
   staging as S1) fused into the DVE drain; bf16 DMA out (host upcasts).

Batch-level software pipeline: S1(b+1) is emitted between mix(b) and
idft(b) so the PE has matmul work while the mix(b) elementwise tail
(shrink/gate chains on DVE/GpSimd) drains; its u1 psum drains are pinned
to the otherwise-idle ACT engine so the pS psum rotation does not queue
behind gate ops in the DVE FIFO.  S2(b+1) follows idft(b).  (Emitting
S2(b+1) before idft(b) and gate-op batching were both tried and measured
SLOWER; the 3-big-XBAR-per-half split interleaved into S2 beats both 34
small XBARs and 1 monolithic one.)

PSUM rule: matmul start=True clears has_written for the WHOLE bank, so each
bank gets exactly one start=True (its first matmul); all later matmuls use
start=False (fresh ranges overwrite, accumulation ranges add).

Hardcoded shapes: x [4,768,128,128] f32, w1/w2 [2,8,96,96], b1/b2 [2,8,96].
"""

import os
import numpy as np
import ml_dtypes

B, C, H, W = 4, 768, 128, 128
NBLK, BS = 8, 96          # spectral blocks, channels per core
WF = 65                   # rfft size along W
LAM = 0.01                # softshrink threshold
NS = 0.1                  # leaky relu negative slope

BF16 = ml_dtypes.bfloat16

LAST_RESULT = {}          # diagnostics (exec_time_ns) for the test harness


def _twiddles():
    n = 128
    wv = np.arange(n)[:, None].astype(np.float64)
    jv = np.arange(n)[None, :].astype(np.float64)
    ang = 2.0 * np.pi * wv * jv / n  # [128,128]

    # S1 moving operand [w, 130]: cols 0..64 cos/n ; cols 65..129 -sin/n
    # (imag cols 65 and 129 i.e. wf=0,64 are exactly zero)
    fw = np.zeros((n, 130), np.float64)
    fw[:, :WF] = np.cos(ang[:, :WF]) / n
    fw[:, WF + 1:WF + 64] = -np.sin(ang[:, 1:64]) / n

    c = np.cos(ang)
    s = np.sin(ang)
    s2cs = np.hstack([c, -s])     # S2 moving for re-stationary: [cos | -sin]
    s2sc = np.hstack([s, c])      # S2 moving for im-stationary: [sin | cos]
    mcs = np.hstack([c, s])       # iDFT moving for Yr: [cos | sin]
    msc = np.hstack([-s, c])      # iDFT moving for Yi: [-sin | cos]

    # final irfft_W stationaries
    alpha = np.full((WF, 1), 2.0)
    alpha[0, 0] = 1.0
    alpha[64, 0] = 1.0
    fic = alpha * np.cos(2.0 * np.pi * np.arange(WF)[:, None]
                         * np.arange(n)[None, :] / n) / n   # [65, 128]
    fis64 = np.zeros((64, n), np.float64)                   # row 0 (wf=0) = 0
    fis64[1:64] = -2.0 * np.sin(2.0 * np.pi * np.arange(1, 64)[:, None]
                                * np.arange(n)[None, :] / n) / n

    return (fw.astype(BF16), s2cs.astype(BF16), s2sc.astype(BF16),
            mcs.astype(BF16), msc.astype(BF16),
            fic.astype(BF16), fis64.astype(BF16))


def _build():
    import concourse.mybir as mybir
    import concourse.tile as tile
    import concourse.bass as _bass
    from concourse import bacc

    dt = mybir.dt
    AF = mybir.ActivationFunctionType
    ALU = mybir.AluOpType

    nc = bacc.Bacc("TRN2", target_bir_lowering=False, debug=False)

    xt = nc.declare_dram_parameter("xt", [B, W, BS, H], dt.bfloat16, isOutput=False)
    # out is [b, c, w, h]; host transposes the last two axes back
    out = nc.declare_dram_parameter("out", [B, BS, W, H], dt.bfloat16, isOutput=True)

    fw_d = nc.declare_dram_parameter("fw", [128, 130], dt.bfloat16, isOutput=False)
    s2cs_d = nc.declare_dram_parameter("s2cs", [128, 256], dt.bfloat16, isOutput=False)
    s2sc_d = nc.declare_dram_parameter("s2sc", [128, 256], dt.bfloat16, isOutput=False)
    mcs_d = nc.declare_dram_parameter("mcs", [128, 256], dt.bfloat16, isOutput=False)
    msc_d = nc.declare_dram_parameter("msc", [128, 256], dt.bfloat16, isOutput=False)
    fic_d = nc.declare_dram_parameter("fic", [WF, 128], dt.bfloat16, isOutput=False)
    fis_d = nc.declare_dram_parameter("fis64", [64, 128], dt.bfloat16, isOutput=False)

    wnames = ("w1ra", "w1ia", "w1in", "w1r_")
    wshapes = {"w1ra": [BS + 1, 128], "w1ia": [BS + 1, 128],
               "w1in": [BS, 128], "w1r_": [BS, 128],
               "w2cat1": [BS + 1, 192], "w2cat2": [BS, 192]}
    wds = {nm: nc.declare_dram_parameter(nm, wshapes[nm], dt.bfloat16,
                                         isOutput=False)
           for nm in list(wnames) + ["w2cat1", "w2cat2"]}

    dbg = bool(int(os.environ.get("AFNO_DEBUG", "0")))
    if dbg:
        d_u1 = nc.declare_dram_parameter("d_u1", [128, BS, 130], dt.bfloat16, isOutput=True)
        d_utr = nc.declare_dram_parameter("d_utr", [BS + 1, 2, WF, 128], dt.bfloat16, isOutput=True)
        d_utT = nc.declare_dram_parameter("d_utT", [128, 2, WF, BS], dt.bfloat16, isOutput=True)
        d_o1 = nc.declare_dram_parameter("d_o1", [BS + 1, 2, 512], dt.bfloat16, isOutput=True)
        d_ybig = nc.declare_dram_parameter("d_ybig", [128, 2, WF, BS], dt.bfloat16, isOutput=True)
        d_vst = nc.declare_dram_parameter("d_vst", [WF, 4, 256], dt.bfloat16, isOutput=True)

    with tile.TileContext(nc) as tc:
        with (
            tc.tile_pool(name="consts", bufs=1) as consts,
            tc.tile_pool(name="xts", bufs=4) as xts_p,
            tc.tile_pool(name="u1", bufs=1) as u1_p,
            tc.tile_pool(name="utr", bufs=1) as utr_p,
            tc.tile_pool(name="utT", bufs=1) as utT_p,
            tc.tile_pool(name="ybig", bufs=1) as ybig_p,
            tc.tile_pool(name="o1", bufs=3) as o1_p,
            tc.tile_pool(name="sg", bufs=3) as sg_p,
            tc.tile_pool(name="vst", bufs=2) as vst_p,
            tc.tile_pool(name="ot", bufs=3) as ot_p,
            tc.tile_pool(name="pS", bufs=2, space="PSUM") as pS,
            tc.tile_pool(name="pM", bufs=2, space="PSUM") as pM,
            tc.tile_pool(name="pN", bufs=2, space="PSUM") as pN,
        ):
            # ---------------- constants ----------------
            def cload(dparam, shape, tag):
                t = consts.tile(shape, dt.bfloat16, tag=tag, name=tag)
                nc.sync.dma_start(out=t[:], in_=dparam[:, :])
                return t

            fw = cload(fw_d, [128, 130], "fw")
            s2cs = cload(s2cs_d, [128, 256], "s2cs")
            s2sc = cload(s2sc_d, [128, 256], "s2sc")
            mcs = cload(mcs_d, [128, 256], "mcs")
            msc = cload(msc_d, [128, 256], "msc")
            fic = cload(fic_d, [WF, 128], "fic")
            fis64 = cload(fis_d, [64, 128], "fis64")
            wt = {nm: cload(wds[nm], wshapes[nm], nm)
                  for nm in list(wnames) + ["w2cat1", "w2cat2"]}

            def load_x(b):
                xh = []
                for hh in range(2):
                    t = xts_p.tile([128, 48, 128], dt.bfloat16, tag="xts", name="xts")
                    nc.sync.dma_start(out=t[:], in_=xt[b, :, hh * 48:(hh + 1) * 48, :])
                    xh.append(t)
                return xh

            def s1(b, xh):
                u1 = u1_p.tile([128, BS, 130], dt.bfloat16, tag="u1")
                for g in range(BS // 3):
                    ps = pS.tile([128, 3, 130], dt.float32, tag="pS", name="ps1")
                    for k in range(3):
                        c = 3 * g + k
                        lhs = xh[c // 48][:, c % 48, :]
                        nc.tensor.matmul(ps[:, k, :], lhs, fw,
                                         start=(k == 0), stop=(k == 2),
                                         skip_group_check=True)
                    # pinned to ACT: during the mix(b) tail the DVE/GpSimd
                    # FIFOs are full of gate ops; ACT is idle there, so S1(b+1)
                    # psum rotation must not queue behind DVE.
                    nc.scalar.activation(u1[:, 3 * g:3 * g + 3, :], ps[:, :, :],
                                         AF.Copy, bias=0.0, scale=1.0)
                if dbg and b == 0:
                    nc.sync.dma_start(out=d_u1[:, :, :], in_=u1[:, :, :])
                return u1

            def s2(b, u1):
                # DFT along H, data-stationary per frequency; uT XBAR
                # transposes issued in 3 big j-groups interleaved in.
                utr = utr_p.tile([BS + 1, 2, WF, 128], dt.bfloat16, tag="utr")
                utT = utT_p.tile([128, 2, WF, BS], dt.bfloat16, tag="utT")
                nc.gpsimd.memset(utr[BS:BS + 1, 0, :, :], 1.0)
                xbar_after = {11: (0, 24), 23: (24, 48), 32: (48, WF)}
                for jp in range(33):
                    js = [j for j in (2 * jp, 2 * jp + 1) if j < WF]
                    ps = pS.tile([128, 2, 256], dt.float32, tag="pS", name="ps2")
                    for q, j in enumerate(js):
                        lr = u1[:, :, j]
                        li = u1[:, :, 65 + j]
                        nc.tensor.matmul(ps[0:BS, q, :], lr, s2cs,
                                         start=(q == 0), stop=False,
                                         skip_group_check=True)
                        nc.tensor.matmul(ps[0:BS, q, :], li, s2sc,
                                         start=False, stop=(q == len(js) - 1),
                                         skip_group_check=True)
                    src = ps[0:BS, 0:len(js), :].rearrange(
                        "p q (r h) -> p q r h", r=2)
                    dst = utr[0:BS, :, js[0]:js[0] + len(js), :].rearrange(
                        "p r j h -> p j r h")
                    nc.any.tensor_copy(dst, src)
                    if jp in xbar_after:
                        lo, hi = xbar_after[jp]
                        for ri in range(2):
                            nc.sync.dma_start_transpose(
                                out=utT[:, ri, lo:hi, :],
                                in_=utr[0:BS, ri, lo:hi, :])
                if dbg and b == 0:
                    nc.sync.dma_start(out=d_utr[:, :, :, :], in_=utr[:, :, :, :])
                return utr, utT

            def mix(b, utr, utT):
                ybig = ybig_p.tile([128, 2, WF, BS], dt.bfloat16, tag="ybig")
                nchunk = 17                   # 16 x 512 + 1 x 128

                def mix1(ci):
                    j0 = 4 * ci
                    njj = min(4, WF - j0)
                    sz = njj * 128
                    ur = utr[0:BS + 1, 0, j0:j0 + njj, :]
                    ui = utr[0:BS, 1, j0:j0 + njj, :]
                    p1 = pM.tile([128, 1024], dt.float32, tag="pM", name="p1")
                    nc.tensor.matmul(p1[:, 0:sz], wt["w1ra"], ur,
                                     start=True, stop=False, skip_group_check=True)
                    nc.tensor.matmul(p1[:, 512:512 + sz], wt["w1ia"], ur,
                                     start=True, stop=False, skip_group_check=True)
                    nc.tensor.matmul(p1[:, 0:sz], wt["w1in"], ui,
                                     start=False, stop=False, skip_group_check=True)
                    nc.tensor.matmul(p1[:, 512:512 + sz], wt["w1r_"], ui,
                                     start=False, stop=True, skip_group_check=True)
                    o1 = o1_p.tile([BS + 1, 2, 512], dt.bfloat16, tag="o1",
                                   name="o1")
                    nc.any.tensor_copy(
                        o1[BS:BS + 1, 0, 0:sz],
                        utr[BS:BS + 1, 0, j0:j0 + njj, :])
                    psrc = p1[0:BS, :].rearrange("p (r f) -> p r f", r=2)[:, :, 0:sz]
                    nc.scalar.activation(o1[0:BS, :, 0:sz], psrc, AF.Prelu,
                                         bias=0.0, scale=1.0, alpha=NS)
                    return (ci, j0, njj, o1)

                def mix2T(st):
                    ci, j0, njj, o1 = st
                    if dbg and b == 0 and ci == 0:
                        nc.sync.dma_start(out=d_o1[:, :, :], in_=o1[:, :, :])
                    for t0 in range(0, njj, 2):
                        js = [jj for jj in (t0, t0 + 1) if jj < njj]
                        p2 = pN.tile([128, 2, 192], dt.float32, tag="pN",
                                     name="p2")
                        for q, jj in enumerate(js):
                            o1r = o1[0:BS + 1, 0, 128 * jj:128 * jj + 128]
                            o1i = o1[0:BS, 1, 128 * jj:128 * jj + 128]
                            nc.tensor.matmul(p2[:, q, :], o1r, wt["w2cat1"],
                                             start=(q == 0), stop=False,
                                             skip_group_check=True)
                            nc.tensor.matmul(p2[:, q, :], o1i, wt["w2cat2"],
                                             start=False, stop=(q == len(js) - 1),
                                             skip_group_check=True)
                        # softshrink: s = p2 - clamp(p2, +-LAM)   [128, 2, 192]
                        nq = len(js)
                        cl = sg_p.tile([128, 2, 192], dt.bfloat16, tag="cl",
                                       name="cl")
                        nc.vector.tensor_scalar(cl[:, 0:nq, :], p2[:, 0:nq, :],
                                                -LAM, LAM, ALU.max, ALU.min)
                        sh = sg_p.tile([128, 2, 192], dt.bfloat16, tag="sh",
                                       name="sh")
                        nc.vector.tensor_sub(sh[:, 0:nq, :], p2[:, 0:nq, :],
                                             cl[:, 0:nq, :])
                        # gate: y = s * u (complex), in [hf, c] layout
                        jg0 = j0 + t0
                        _s = sh[:, :, :]
                        sr2 = _bass.AP(
                            tensor=_s.tensor, offset=_s.offset,
                            ap=[_s.ap[0], [192, nq], [0, 2], [1, BS]])
                        si2 = _bass.AP(
                            tensor=_s.tensor, offset=_s.offset + 96,
                            ap=[_s.ap[0], [192, nq], [0, 2], [1, BS]])
                        u2 = utT[:, :, jg0:jg0 + nq, :].rearrange(
                            "p r j c -> p j r c")
                        ta = sg_p.tile([128, 2, 2, BS], dt.bfloat16, tag="ta",
                                       name="ta")
                        tb = sg_p.tile([128, 2, 2, BS], dt.bfloat16, tag="tb",
                                       name="tb")
                        nc.vector.tensor_mul(ta[:, 0:nq, :, :], sr2, u2)
                        nc.gpsimd.tensor_mul(tb[:, 0:nq, :, :], si2, u2)
                        nc.vector.tensor_sub(
                            ybig[:, 0, jg0:jg0 + nq, :],
                            ta[:, 0:nq, 0, :], tb[:, 0:nq, 1, :])
                        nc.gpsimd.tensor_add(
                            ybig[:, 1, jg0:jg0 + nq, :],
                            ta[:, 0:nq, 1, :], tb[:, 0:nq, 0, :])

                prev = None
                for ci in range(nchunk):
                    cur = mix1(ci)
                    if prev is not None:
                        mix2T(prev)
                    prev = cur
                mix2T(prev)
                if dbg and b == 0:
                    nc.sync.dma_start(out=d_ybig[:, :, :, :], in_=ybig[:, :, :, :])
                return ybig

            def idft_final(b, ybig, xh):
                def idftA(p):
                    pV = pN.tile([128, 2, 256], dt.float32, tag="pN", name="pV")
                    for cc in range(2):
                        c = 2 * p + cc
                        yr = ybig[:, 0, :, c]
                        yi = ybig[:, 1, :, c]
                        nc.tensor.matmul(pV[0:WF, cc, :], yr, mcs,
                                         start=(cc == 0), stop=False,
                                         skip_group_check=True)
                        nc.tensor.matmul(pV[0:WF, cc, :], yi, msc,
                                         start=False, stop=(cc == 1),
                                         skip_group_check=True)
                    return pV

                def vdrain(p, pV, vst):
                    q = p % 2
                    nc.any.tensor_copy(vst[0:WF, 2 * q:2 * q + 2, :],
                                       pV[0:WF, :, :])

                def final(qd, vst):
                    pO = pM.tile([128, 4, 128], dt.float32, tag="pM", name="pO")
                    nc.tensor.matmul(pO[:, :, :], fic, vst[0:WF, :, 0:128],
                                     start=True, stop=False,
                                     skip_group_check=True)
                    nc.tensor.matmul(pO[:, :, :], fis64, vst[0:64, :, 128:256],
                                     start=False, stop=True,
                                     skip_group_check=True)
                    c0 = 4 * qd
                    hh = c0 // 48
                    xh4 = xh[hh][:, c0 % 48:c0 % 48 + 4, :]
                    ot = ot_p.tile([128, 4, 128], dt.bfloat16, tag="ot",
                                   name="ot")
                    nc.vector.tensor_add(
                        ot[:, :, :].rearrange("p c f -> p (c f)"),
                        pO[:, :, :].rearrange("p c f -> p (c f)"),
                        xh4.rearrange("p c f -> p (c f)"))
                    nc.sync.dma_start(
                        out=out[b, c0:c0 + 4, :, :].rearrange("c w h -> w c h"),
                        in_=ot[:, :, :])

                vsts = {}
                for p in range(BS // 2):
                    qd = p // 2
                    if p % 2 == 0:
                        vsts[qd] = vst_p.tile([WF, 4, 256], dt.bfloat16,
                                              tag="vst", name="vst")
                    pV = idftA(p)
                    vdrain(p, pV, vsts[qd])
                    if dbg and b == 0 and p == 1:
                        nc.sync.dma_start(out=d_vst[:, :, :], in_=vsts[0][:, :, :])
                    if p % 2 == 1 and p >= 3:
                        final(qd - 1, vsts[qd - 1])
                final(BS // 4 - 1, vsts[BS // 4 - 1])

            # ---- batch-level software pipeline: S1(b+1) fills the PE while
            # ---- the mix(b) elementwise tail drains; S2(b+1) follows idft(b).
            xh_all = {0: load_x(0)}
            u1 = s1(0, xh_all[0])
            utr, utT = s2(0, u1)
            for b in range(B):
                if b + 1 < B:
                    xh_all[b + 1] = load_x(b + 1)
                ybig = mix(b, utr, utT)
                if b + 1 < B:
                    u1 = s1(b + 1, xh_all[b + 1])
                idft_final(b, ybig, xh_all[b])
                del xh_all[b]
                if b + 1 < B:
                    utr, utT = s2(b + 1, u1)

    nc.finalize()
    return nc


_BUILT = None


def _get_built():
    global _BUILT
    if _BUILT is None:
        _BUILT = _build()
    return _BUILT


def _make_in_maps(x, w1, b1, w2, b2):
    fw, s2cs, s2sc, mcs, msc, fic, fis64 = _twiddles()
    in_maps = []
    for k in range(NBLK):
        xs = x[:, k * BS:(k + 1) * BS]
        w1r, w1i = w1[0, k], w1[1, k]
        w2r, w2i = w2[0, k], w2[1, k]

        def pad128(a):
            o = np.zeros((a.shape[0], 128), np.float32)
            o[:, 0:BS] = a
            return o.astype(BF16)

        w2cat1 = np.vstack([np.hstack([w2r, w2i]),
                            np.hstack([b2[0, k][None, :], b2[1, k][None, :]])])
        w2cat2 = np.hstack([-w2i, w2r])
        m = {
            "xt": np.ascontiguousarray(xs.transpose(0, 3, 1, 2)).astype(BF16),
            "fw": fw, "s2cs": s2cs, "s2sc": s2sc, "mcs": mcs, "msc": msc,
            "fic": fic, "fis64": fis64,
            "w1ra": pad128(np.vstack([w1r, b1[0, k][None, :]])),
            "w1ia": pad128(np.vstack([w1i, b1[1, k][None, :]])),
            "w1in": pad128(-w1i), "w1r_": pad128(w1r),
            "w2cat1": w2cat1.astype(BF16), "w2cat2": w2cat2.astype(BF16),
        }
        in_maps.append(m)
    return in_maps


def kernel(x, w1, b1, w2, b2):
    from concourse.bass_utils import run_bass_kernel_spmd

    nc = _get_built()
    in_maps = _make_in_maps(x, w1, b1, w2, b2)

    trace = bool(int(os.environ.get("AFNO_TRACE", "0")))
    kw = {}
    if trace:
        import tempfile
        kw["tmpdir"] = tempfile.mkdtemp(prefix="afno_trace_")
        LAST_RESULT["trace_dir"] = kw["tmpdir"]
    res = run_bass_kernel_spmd(nc, in_maps, core_ids=list(range(NBLK)),
                               trace=trace, **kw)
    LAST_RESULT["exec_time_ns"] = res.exec_time_ns
    LAST_RESULT["results"] = res.results

    outp = np.empty((B, C, H, W), np.float32)
    for k in range(NBLK):
        outp[:, k * BS:(k + 1) * BS] = \
            res.results[k]["out"].astype(np.float32).transpose(0, 1, 3, 2)
    return outp
